# revision 26
# baseline (speedup 1.0000x reference)
"""Causal self-attention on 8 TRN2 NeuronCores.

Sharding: pure data-parallel on batch (B=8 -> one batch element per core,
no collectives). Each core computes its full [T, C] output slice.

Per-core dataflow (all matmuls bf16 with fp32 PSUM accumulation):
  xT [C,T] (host-pretransposed) --+--> qkT = (w_qk^T @ xT) + b_qk  [2C, T]
                                  +--> V   = (xT^T @ w_v) + b_v    [T, C]
                                       (per head: [v_h | ones], 128 cols)
  per head h, per q-chunk (512):
    S^T[k,q] = k_h . q_h           (lhsT = kT_h slice, rhs = qT_h slice)
    E = exp(S^T)                   (ACT, PSUM->SBUF, bf16 out; q pre-scaled
                                    by 1/8 on host so no separate scale op)
    E *= causal mask               (one full-block mul over the DIAG tiles)
    [O'[d,q]; s[q] x64] = [v_h | 1]^T @ E  (ones cols of Vp replicate the
                                    softmax sums on partitions 64:128)
    r = 1/s (DVE, 64 rows) ; Y^T block = O' * r
  Z = (Y^T)^T @ w_proj + b_proj    -> DMA PSUM -> DRAM out
Schedule: attention runs q-chunk-outer with the output projection for each
q-chunk interleaved right after it, so proj matmuls keep the PE busy while
the next q-chunk's exp runs on ACT. Biases enter as K=1 rank-1 accumulate
matmuls (v/proj) or a DVE scalar-add (qk); zero biases compile out.
"""

import os
import sys
from contextlib import ExitStack

import numpy as np

try:
    import ml_dtypes
except ImportError:  # pragma: no cover
    sys.path.insert(0, "/opt/trn_rl_repo")
    import ml_dtypes

BF16 = ml_dtypes.bfloat16

B, T, C = 8, 1024, 1024
H, HD = 16, 64
N_CORES = 8

# Toggled by test harness to capture a hardware profile.
TRACE = False
LAST_EXEC_NS = None
LAST_RESULTS = None

_NC_CACHE = {}


RECIP_MODE = "approx_psum"  # "approx_psum" | "approx_sbuf" | "lnexp"
# Vp column order: "v_ones" puts V in cols 0:HD and ones in HD:2HD (sums on
# partitions HD:); "ones_v" flips them so the sums land on partitions 0:HD,
# base-aligned with the rrow tile for the PSUM-direct reciprocal.
VP_ORDER = "ones_v"


def _build_nc(Tp, Cp, Hp, HDp, reps=1, with_bias_qk=True, with_bias_v=True,
              with_bias_p=True, recip_mode=None, vp_order=None):
    if recip_mode is None:
        recip_mode = RECIP_MODE
    if vp_order is None:
        vp_order = VP_ORDER
    ones_first = vp_order == "ones_v"
    vsl_vp = slice(HDp, 2 * HDp) if ones_first else slice(0, HDp)
    ssl_pav = slice(0, HDp) if ones_first else slice(HDp, 2 * HDp)
    osl_pav = slice(HDp, 2 * HDp) if ones_first else slice(0, HDp)
    import concourse.bass as bass
    import concourse.tile as tile
    from concourse import bacc, mybir

    bf = mybir.dt.bfloat16
    f32 = mybir.dt.float32
    AF = mybir.ActivationFunctionType

    P = 128
    CT = Cp // P            # c-tiles (contraction tiles)
    TT = Tp // P            # t-tiles
    QC = min(512, Tp)       # q-chunk width (free dim per matmul)
    NQ = Tp // QC           # q-chunks
    TCH = min(512, Tp)      # t-chunk width for qkT rhs
    TJ = Tp // TCH
    DIAG = QC // P          # diagonal k-tiles per q-chunk
    M2C = 2 * Cp // P       # qk m-chunks
    VJ = Cp // QC           # v/proj column chunks

    nc = bacc.Bacc("TRN2", target_bir_lowering=False, debug=False)

    xT_d = nc.declare_dram_parameter("xT", [Cp, Tp], bf, isOutput=False)
    wqk_d = nc.declare_dram_parameter("w_qk", [Cp, 2 * Cp], bf, isOutput=False)
    wv_d = nc.declare_dram_parameter("w_v", [Cp, Cp], bf, isOutput=False)
    wp_d = nc.declare_dram_parameter("w_proj", [Cp, Cp], bf, isOutput=False)
    bqk_d = nc.declare_dram_parameter("b_qk", [M2C, P], f32, isOutput=False)
    bv_d = nc.declare_dram_parameter("b_v", [1, Cp], bf, isOutput=False)
    bp_d = nc.declare_dram_parameter("b_proj", [1, Cp], bf, isOutput=False)
    mask_d = nc.declare_dram_parameter("masks", [DIAG, P, QC], bf, isOutput=False)
    out_d = nc.declare_dram_parameter("out", [Tp, Cp], f32, isOutput=True)

    with tile.TileContext(nc) as tc, ExitStack() as ctx:
        consts = ctx.enter_context(tc.tile_pool(name="consts", bufs=1))
        epool = ctx.enter_context(tc.tile_pool(name="epool", bufs=3))
        rpool = ctx.enter_context(tc.tile_pool(name="rpool", bufs=2))
        psum1 = ctx.enter_context(tc.tile_pool(name="psum1", bufs=4, space="PSUM"))
        psum_s = ctx.enter_context(tc.tile_pool(name="psum_s", bufs=2, space="PSUM"))

        # ---- persistent SBUF buffers ----
        xT = consts.tile([P, CT, Tp], bf)
        wqk = consts.tile([P, CT, 2 * Cp], bf)
        wv = consts.tile([P, CT, Cp], bf)
        wp = consts.tile([P, CT, Cp], bf)
        qkT = consts.tile([P, M2C, Tp], bf)
        # Vp cols 0:HD hold V; cols HD:2HD hold ones, so the AV matmul lands
        # the softmax sums replicated on partitions HD..2HD-1 (no separate
        # PE broadcast needed for the 1/sum normalization).
        Vp = consts.tile([P, TT, Hp, 2 * HDp], bf)
        YT = consts.tile([P, CT, Tp], bf)
        bqk = consts.tile([P, M2C], f32)
        bv = bp = None
        if with_bias_v:
            bv = consts.tile([1, Cp], bf)
        if with_bias_p:
            bp = consts.tile([1, Cp], bf)
        ones = consts.tile([1, max(QC, P)], bf)
        masks = consts.tile([P, DIAG, QC], bf)

        nc.sync.dma_start(xT[:], xT_d.rearrange("(ct p) t -> p ct t", p=P))
        nc.sync.dma_start(wqk[:], wqk_d.rearrange("(ct p) n -> p ct n", p=P))
        nc.sync.dma_start(wv[:], wv_d.rearrange("(ct p) n -> p ct n", p=P))
        nc.sync.dma_start(wp[:], wp_d.rearrange("(ct p) n -> p ct n", p=P))
        nc.sync.dma_start(bqk[:], bqk_d.rearrange("m p -> p m"))
        if with_bias_v:
            nc.sync.dma_start(bv[:], bv_d[:])
        if with_bias_p:
            nc.sync.dma_start(bp[:], bp_d[:])
        nc.sync.dma_start(masks[:], mask_d.rearrange("r p q -> p r q"))
        nc.gpsimd.memset(ones[:], 1.0)
        nc.gpsimd.memset(Vp[:], 1.0)  # ones columns survive; V region overwritten

        def _emit_qk_chunk(m):
            # qkT[m] = (w_qk^T @ xT)[m] + b_qk : bf16 copyback (DVE)
            msl = slice(m * P, (m + 1) * P)
            for tj in range(TJ):
                tsl = slice(tj * TCH, (tj + 1) * TCH)
                ps = psum1.tile([P, TCH], f32, tag="ps_mm")
                for ct in range(CT):
                    nc.tensor.matmul(
                        ps[:], lhsT=wqk[:, ct, msl], rhs=xT[:, ct, tsl],
                        start=(ct == 0), stop=(ct == CT - 1),
                    )
                if with_bias_qk:
                    nc.vector.tensor_scalar_add(
                        qkT[:, m, tsl], ps[:], bqk[:, m:m + 1],
                    )
                else:
                    nc.scalar.copy(out=qkT[:, m, tsl], in_=ps[:])

        def _emit_v_chunk(ti):
            # V rows [128*ti, 128*(ti+1)) interleaved into Vp[.., ti, h, 0:HD]
            tsl = slice(ti * P, (ti + 1) * P)
            for vj in range(VJ):
                vsl = slice(vj * QC, (vj + 1) * QC)
                ps = psum1.tile([P, QC], f32, tag="ps_mm")
                for ct in range(CT):
                    nc.tensor.matmul(
                        ps[:], lhsT=xT[:, ct, tsl], rhs=wv[:, ct, vsl],
                        start=(ct == 0),
                        stop=(ct == CT - 1 and not with_bias_v),
                    )
                if with_bias_v:
                    nc.tensor.matmul(
                        ps[:], lhsT=ones[0:1, 0:P], rhs=bv[0:1, vsl],
                        start=False, stop=True,
                    )
                hpc = QC // HDp  # heads per chunk
                nc.vector.tensor_copy(
                    out=Vp[:, ti, vj * hpc:(vj + 1) * hpc, vsl_vp],
                    in_=ps[:].rearrange("p (h d) -> p h d", d=HDp),
                )

        def _emit_attn_unit(u, qj):
            # Head pair: head 2u on partitions 0:64, head 2u+1 on 64:128 of
            # qkT chunk u (q) / M2C//2+u (k). The A/B matmuls use disjoint PE
            # row groups (tile_position auto-derived from base_partition), so
            # they run concurrently in the array.
            qk_parts = (
                (qkT[0:HDp, u, :], qkT[0:HDp, M2C // 2 + u, :]),
                (qkT[HDp:P, u, :], qkT[HDp:P, M2C // 2 + u, :]),
            )
            nk = DIAG * (qj + 1)  # active k-tiles (causal)
            q0 = qj * QC
            E_A = epool.tile([P, DIAG * NQ, QC], bf, tag="E")
            E_B = epool.tile([P, DIAG * NQ, QC], bf, tag="E")
            for g in range(nk // 2):
                offg = max(0, P * (2 * g - DIAG * qj))
                ps_h = [
                    psum_s.tile([P, 2, QC], f32, tag="ps_s",
                                name=f"ps_s_{u}_{qj}_{g}_{hh}")
                    for hh in range(2)
                ]
                for r2 in range(2):
                    ki = 2 * g + r2
                    ksl = slice(ki * P, (ki + 1) * P)
                    for half, (qT, kT) in enumerate(qk_parts):
                        nc.tensor.matmul(
                            ps_h[half][:, r2, offg:],
                            lhsT=kT[:, ksl],
                            rhs=qT[:, q0 + offg:q0 + QC],
                            start=True, stop=True,
                        )
                gsl = slice(2 * g, 2 * g + 2)
                nc.scalar.activation(
                    E_A[:, gsl, offg:], ps_h[0][:, :, offg:], AF.Exp)
                nc.scalar.activation(
                    E_B[:, gsl, offg:], ps_h[1][:, :, offg:], AF.Exp)
            # causal mask on the DIAG diagonal tiles (the last ones)
            for rel in range(DIAG):
                ki = DIAG * qj + rel
                off = P * rel
                for E in (E_A, E_B):
                    nc.vector.tensor_mul(
                        out=E[:, ki, off:], in0=E[:, ki, off:],
                        in1=masks[:, rel, off:],
                    )
            # O'[d,q] on partitions 0:HD; softmax sums replicated on
            # partitions HD:2HD via the ones columns of Vp
            pav_A = psum1.tile([P, QC], f32, tag="ps_mm")
            pav_B = psum1.tile([P, QC], f32, tag="ps_mm")
            # sequential per-head chains: consecutive MMs keep the
            # same PSUM bank (alternating banks per-MM causes PE
            # micro-idles / HAM oscillation)
            for pav, E, h in (
                (pav_A, E_A, 2 * u), (pav_B, E_B, 2 * u + 1),
            ):
                for ki in range(nk):
                    off = max(0, P * (ki - DIAG * qj))
                    nc.tensor.matmul(
                        pav[:, off:],
                        lhsT=Vp[:, ki, h, :], rhs=E[:, ki, off:],
                        start=(ki == 0), stop=(ki == nk - 1),
                    )
            # 1/sum on the replicated-sums partitions (64 identical lanes);
            # the iterative DVE reciprocal is ~8 cycles/elem and would
            # dominate the kernel, so use a fast path instead.
            qsl = slice(q0, q0 + QC)
            for pav, half in ((pav_A, 0), (pav_B, 1)):
                rrow = rpool.tile([HDp, QC], f32, tag="rrow")
                if recip_mode == "approx_psum":
                    nc.vector.reciprocal_approx_fast(
                        out=rrow[:], in_=pav[ssl_pav, :])
                elif recip_mode == "approx_sbuf":
                    srow = rpool.tile([HDp, QC], f32, tag="srow")
                    nc.vector.tensor_copy(out=srow[:], in_=pav[ssl_pav, :])
                    nc.vector.reciprocal_approx_fast(out=rrow[:], in_=srow[:])
                elif recip_mode == "lnexp":
                    # r = exp(-ln(s)); Ln and Exp share one ACT table set
                    # (natural_log_exp_and_others), so no extra table loads.
                    srow = rpool.tile([HDp, QC], f32, tag="srow")
                    nc.scalar.activation(srow[:], pav[ssl_pav, :], AF.Ln)
                    nc.scalar.activation(rrow[:], srow[:], AF.Exp, scale=-1.0)
                else:
                    raise ValueError(recip_mode)
                ysl = slice(0, HDp) if half == 0 else slice(HDp, P)
                nc.vector.tensor_mul(
                    out=YT[ysl, u, qsl], in0=pav[osl_pav, :], in1=rrow[:],
                )

        def _emit_proj_chunk(ti):
            # Z rows [128*ti, ..) = Y @ w_proj + b_proj -> DRAM
            tsl = slice(ti * P, (ti + 1) * P)
            for zj in range(VJ):
                zsl = slice(zj * QC, (zj + 1) * QC)
                ps = psum1.tile([P, QC], f32, tag="ps_mm")
                for ct in range(CT):
                    nc.tensor.matmul(
                        ps[:], lhsT=YT[:, ct, tsl], rhs=wp[:, ct, zsl],
                        start=(ct == 0),
                        stop=(ct == CT - 1 and not with_bias_p),
                    )
                if with_bias_p:
                    nc.tensor.matmul(
                        ps[:], lhsT=ones[0:1, 0:P], rhs=bp[0:1, zsl],
                        start=False, stop=True,
                    )
                zt = rpool.tile([P, QC], f32, tag="zt")
                nc.scalar.copy(out=zt[:], in_=ps[:])
                nc.sync.dma_start(out_d[tsl, zsl], zt[:])

        def _emit_body():
            # qkv: emit the q/k chunks for the first attention units early,
            # then V (needed by the first AV), then the rest.
            for u in (0, 1):
                _emit_qk_chunk(u)
                _emit_qk_chunk(M2C // 2 + u)
            for ti in range(TT // 2):
                _emit_v_chunk(ti)
            for u in range(2, Hp // 2):
                _emit_qk_chunk(u)
                _emit_qk_chunk(M2C // 2 + u)
            for ti in range(TT // 2, TT):
                _emit_v_chunk(ti)
            # attention u-outer: each head pair does its small (qj=0,
            # DVE-lean) and large (qj=1, PE-heavy) chunks back to back so
            # the engines stay load-balanced. proj for the first q-half is
            # emitted as soon as its last YT slice exists, giving the
            # scheduler dense PE work while the final unit's exp/normalize
            # run on ACT/DVE.
            for u in range(Hp // 2):
                _emit_attn_unit(u, 0)
                if u == Hp // 2 - 1:
                    for ti in range(QC // P):
                        _emit_proj_chunk(ti)
                _emit_attn_unit(u, 1)
            for ti in range(QC // P, TT):
                _emit_proj_chunk(ti)

        if reps == 1:
            _emit_body()
        else:
            hint = (
                mybir.EngineType.PE,
                mybir.EngineType.DVE,
                mybir.EngineType.Activation,
            )
            with tc.For_i(0, reps, 1, hint_engines=hint):
                _emit_body()

    nc.finalize()
    return nc


def _prep_shared(w_attn, b_attn, w_proj, b_proj):
    """Host-side layout marshalling of the replicated weights (bf16 cast,
    per-head q/k/v column gather, exact 1/8 q pre-scale)."""
    wr = np.asarray(w_attn, np.float32).reshape(C, H, 3, HD)
    w_q = (wr[:, :, 0, :] * np.float32(0.125)).reshape(C, C)
    w_k = wr[:, :, 1, :].reshape(C, C)
    w_qk = np.ascontiguousarray(
        np.concatenate([w_q, w_k], axis=1)
    ).astype(BF16)
    w_v = np.ascontiguousarray(wr[:, :, 2, :].reshape(C, C)).astype(BF16)

    br = np.asarray(b_attn, np.float32).reshape(H, 3, HD)
    # per-partition column layout for the qkT copyback bias: [M2C, 128] f32
    b_qk = np.ascontiguousarray(
        np.concatenate(
            [(br[:, 0, :] * np.float32(0.125)).reshape(C), br[:, 1, :].reshape(C)]
        ).reshape(2 * C // 128, 128)
    )
    b_v = np.ascontiguousarray(br[:, 2, :].reshape(1, C)).astype(BF16)

    wp = np.ascontiguousarray(np.asarray(w_proj, np.float32)).astype(BF16)
    bp = np.ascontiguousarray(np.asarray(b_proj, np.float32).reshape(1, C)).astype(BF16)

    QCv = min(512, T)
    DIAGv = QCv // 128
    k_idx = np.arange(128)[:, None]
    q_idx = np.arange(QCv)[None, :]
    masks = np.stack(
        [(128 * r + k_idx <= q_idx) for r in range(DIAGv)]
    ).astype(BF16)
    return w_qk, w_v, wp, b_qk, b_v, bp, masks


class _Runner:
    """Cached jit(shard_map) executor for a prebuilt Bass module across
    N cores — same lowering as bass2jax.run_bass_via_pjrt, but reusable
    across calls so warm executions can be timed."""

    def __init__(self, nc, n_cores):
        import jax
        import numpy as _np
        from jax.sharding import Mesh, PartitionSpec
        try:
            from jax.experimental.shard_map import shard_map
        except ImportError:
            from jax.shard_map import shard_map
        from concourse import bass2jax, mybir

        bass2jax.install_neuronx_cc_hook()
        assert not nc.dbg_callbacks
        self.dbg_name = nc.dbg_addr.name if nc.dbg_addr is not None else None
        partition_name = (
            nc.partition_id_tensor.name if nc.partition_id_tensor else None
        )

        in_names, out_names, out_avals = [], [], []
        for alloc in nc.m.functions[0].allocations:
            if not isinstance(alloc, mybir.MemoryLocationSet):
                continue
            name = alloc.memorylocations[0].name
            if alloc.kind == "ExternalInput":
                if name != partition_name:
                    in_names.append(name)
            elif alloc.kind == "ExternalOutput":
                out_names.append(name)
                out_avals.append(
                    jax.core.ShapedArray(
                        tuple(alloc.tensor_shape), mybir.dt.np(alloc.dtype)
                    )
                )
        self.n_params = len(in_names)
        self.in_names = list(in_names)
        self.out_names = out_names
        self.out_avals = out_avals
        self.n_cores = n_cores
        all_names = in_names + out_names
        if partition_name is not None:
            all_names = all_names + [partition_name]

        def _body(*args):
            operands = list(args)
            if partition_name is not None:
                operands.append(bass2jax.partition_id_tensor())
            outs = bass2jax._bass_exec_p.bind(
                *operands,
                out_avals=tuple(out_avals),
                in_names=tuple(all_names),
                out_names=tuple(out_names),
                lowering_input_output_aliases=(),
                sim_require_finite=True,
                sim_require_nnan=True,
                nc=nc,
            )
            return tuple(outs)

        devices = jax.devices()[:n_cores]
        mesh = Mesh(_np.asarray(devices), ("core",))
        n_outs = len(out_names)
        # No donation: the kernel writes every element of every output, so
        # the zero "output seed" operands can live on device and be reused
        # across timed calls.
        self.jitted = jax.jit(
            shard_map(
                _body,
                mesh=mesh,
                in_specs=(PartitionSpec("core"),) * (self.n_params + n_outs),
                out_specs=(PartitionSpec("core"),) * n_outs,
                check_rep=False,
            ),
            keep_unused=True,
        )
        from jax.sharding import NamedSharding

        self.sharding = NamedSharding(mesh, PartitionSpec("core"))
        self.dev_zeros = [
            jax.device_put(
                _np.zeros((n_cores * a.shape[0], *a.shape[1:]), a.dtype),
                self.sharding,
            )
            for a in out_avals
        ]

    def prep_args(self, in_maps):
        import jax
        import numpy as _np

        if self.dbg_name is not None:
            # 8-byte PA as uint32[1,2]; zero -> debugger store+halt skipped
            dbg = _np.zeros((1, 2), _np.uint32)
            in_maps = [{**m, self.dbg_name: dbg} for m in in_maps]
        return [
            jax.device_put(
                _np.concatenate(
                    [_np.asarray(m[name]) for m in in_maps], axis=0
                ),
                self.sharding,
            )
            for name in self.in_names
        ]

    def run(self, concat_in):
        import jax

        out = self.jitted(*concat_in, *self.dev_zeros)
        return jax.block_until_ready(out)

    def results(self, out_arrs):
        import numpy as _np

        return [
            {
                name: _np.asarray(out_arrs[i]).reshape(
                    self.n_cores, *self.out_avals[i].shape
                )[c]
                for i, name in enumerate(self.out_names)
            }
            for c in range(self.n_cores)
        ]


_RUNNER_CACHE = {}


def _get_runner(reps=1, with_bias_qk=True, with_bias_v=True, with_bias_p=True):
    key = (T, C, H, HD, reps, with_bias_qk, with_bias_v, with_bias_p,
           RECIP_MODE, VP_ORDER)
    if key not in _RUNNER_CACHE:
        if key not in _NC_CACHE:
            _NC_CACHE[key] = _build_nc(
                T, C, H, HD, reps=reps, with_bias_qk=with_bias_qk,
                with_bias_v=with_bias_v, with_bias_p=with_bias_p,
            )
        _RUNNER_CACHE[key] = _Runner(_NC_CACHE[key], N_CORES)
    return _RUNNER_CACHE[key]


def _make_in_maps(x, w_attn, b_attn, w_proj, b_proj):
    w_qk, w_v, wp, b_qk, b_v, bp, masks = _prep_shared(
        w_attn, b_attn, w_proj, b_proj
    )
    x = np.asarray(x, np.float32)
    in_maps = []
    for i in range(N_CORES):
        xT = np.ascontiguousarray(x[i].T).astype(BF16)
        in_maps.append({
            "xT": xT, "w_qk": w_qk, "w_v": w_v, "w_proj": wp,
            "b_qk": b_qk, "b_v": b_v, "b_proj": bp, "masks": masks,
        })
    return in_maps


def _bias_flags(b_attn, b_proj):
    br = np.asarray(b_attn, np.float32).reshape(H, 3, HD)
    return (
        bool(br[:, 0:2, :].any()),
        bool(br[:, 2, :].any()),
        bool(np.asarray(b_proj, np.float32).any()),
    )


def kernel(x, w_attn, b_attn, w_proj, b_proj):
    wbqk, wbv, wbp = _bias_flags(b_attn, b_proj)
    runner = _get_runner(with_bias_qk=wbqk, with_bias_v=wbv, with_bias_p=wbp)
    concat_in = runner.prep_args(
        _make_in_maps(x, w_attn, b_attn, w_proj, b_proj)
    )
    res = runner.results(runner.run(concat_in))
    return np.stack([res[i]["out"] for i in range(N_CORES)]).astype(np.float32)


def measure_pair(x, w_attn, b_attn, w_proj, b_proj, iters=12, reps=50):
    """Interleaved warm wall-clock times (s) for reps=1 and reps=R bodies.

    Returns (t1, tR) lists. Interleaving makes the pairwise delta robust
    to slow drift in the axon dispatch RTT."""
    import time

    wbqk, wbv, wbp = _bias_flags(b_attn, b_proj)
    r1 = _get_runner(reps=1, with_bias_qk=wbqk, with_bias_v=wbv,
                     with_bias_p=wbp)
    rR = _get_runner(reps=reps, with_bias_qk=wbqk, with_bias_v=wbv,
                     with_bias_p=wbp)
    in1 = r1.prep_args(_make_in_maps(x, w_attn, b_attn, w_proj, b_proj))
    inR = rR.prep_args(_make_in_maps(x, w_attn, b_attn, w_proj, b_proj))
    r1.run(in1)
    rR.run(inR)
    t1, tR = [], []
    for _ in range(iters):
        t0 = time.perf_counter()
        r1.run(in1)
        t1.append(time.perf_counter() - t0)
        t0 = time.perf_counter()
        rR.run(inR)
        tR.append(time.perf_counter() - t0)
    return t1, tR


def measure(x, w_attn, b_attn, w_proj, b_proj, iters=5, reps=1):
    """Warm wall-clock times (s) of the sharded on-device execution.

    reps > 1 uses a kernel variant whose compute body runs `reps` times
    per dispatch (device-side loop), so per-iteration HW time can be
    resolved despite the ~90 ms axon round-trip overhead."""
    import time

    wbqk, wbv, wbp = _bias_flags(b_attn, b_proj)
    runner = _get_runner(
        reps=reps, with_bias_qk=wbqk, with_bias_v=wbv, with_bias_p=wbp,
    )
    concat_in = runner.prep_args(
        _make_in_maps(x, w_attn, b_attn, w_proj, b_proj)
    )
    runner.run(concat_in)  # warm-up / compile
    times = []
    for _ in range(iters):
        t0 = time.perf_counter()
        runner.run(concat_in)
        times.append(time.perf_counter() - t0)
    return times



# revision 27
# speedup vs baseline: 1.3714x; 1.3714x over previous
"""Causal self-attention on 8 TRN2 NeuronCores.

Sharding: pure data-parallel on batch (B=8 -> one batch element per core,
no collectives). Each core computes its full [T, C] output slice.

Per-core dataflow (all matmuls bf16 with fp32 PSUM accumulation):
  xT [C,T] (host-pretransposed) --+--> qkT = (w_qk^T @ xT) + b_qk  [2C, T]
                                  +--> V   = (xT^T @ w_v) + b_v    [T, C]
                                       (per head: [v_h | ones], 128 cols)
  per head h, per q-chunk (512):
    S^T[k,q] = k_h . q_h           (lhsT = kT_h slice, rhs = qT_h slice)
    E = exp(S^T)                   (ACT, PSUM->SBUF, bf16 out; q pre-scaled
                                    by 1/8 on host so no separate scale op)
    E *= causal mask               (one full-block mul over the DIAG tiles)
    [O'[d,q]; s[q] x64] = [v_h | 1]^T @ E  (ones cols of Vp replicate the
                                    softmax sums on partitions 64:128)
    r = 1/s (DVE, 64 rows) ; Y^T block = O' * r
  Z = (Y^T)^T @ w_proj + b_proj    -> DMA PSUM -> DRAM out
Schedule: attention runs q-chunk-outer with the output projection for each
q-chunk interleaved right after it, so proj matmuls keep the PE busy while
the next q-chunk's exp runs on ACT. Biases enter as K=1 rank-1 accumulate
matmuls (v/proj) or a DVE scalar-add (qk); zero biases compile out.
"""

import os
import sys
from contextlib import ExitStack

import numpy as np

try:
    import ml_dtypes
except ImportError:  # pragma: no cover
    sys.path.insert(0, "/opt/trn_rl_repo")
    import ml_dtypes

BF16 = ml_dtypes.bfloat16

B, T, C = 8, 1024, 1024
H, HD = 16, 64
N_CORES = 8

# Toggled by test harness to capture a hardware profile.
TRACE = False
LAST_EXEC_NS = None
LAST_RESULTS = None

_NC_CACHE = {}


RECIP_MODE = "approx_psum"  # "approx_psum" | "approx_sbuf" | "lnexp"
# Vp column order: "v_ones" puts V in cols 0:HD and ones in HD:2HD (sums on
# partitions HD:); "ones_v" flips them so the sums land on partitions 0:HD,
# base-aligned with the rrow tile for the PSUM-direct reciprocal.
VP_ORDER = "ones_v"


def _build_nc(Tp, Cp, Hp, HDp, reps=1, with_bias_qk=True, with_bias_v=True,
              with_bias_p=True, recip_mode=None, vp_order=None):
    if recip_mode is None:
        recip_mode = RECIP_MODE
    if vp_order is None:
        vp_order = VP_ORDER
    ones_first = vp_order == "ones_v"
    vsl_vp = slice(HDp, 2 * HDp) if ones_first else slice(0, HDp)
    ssl_pav = slice(0, HDp) if ones_first else slice(HDp, 2 * HDp)
    osl_pav = slice(HDp, 2 * HDp) if ones_first else slice(0, HDp)
    import concourse.bass as bass
    import concourse.tile as tile
    from concourse import bacc, mybir

    bf = mybir.dt.bfloat16
    f32 = mybir.dt.float32
    AF = mybir.ActivationFunctionType

    P = 128
    CT = Cp // P            # c-tiles (contraction tiles)
    TT = Tp // P            # t-tiles
    QC = min(512, Tp)       # q-chunk width (free dim per matmul)
    NQ = Tp // QC           # q-chunks
    TCH = min(512, Tp)      # t-chunk width for qkT rhs
    TJ = Tp // TCH
    DIAG = QC // P          # diagonal k-tiles per q-chunk
    M2C = 2 * Cp // P       # qk m-chunks
    VJ = Cp // QC           # v/proj column chunks

    nc = bacc.Bacc("TRN2", target_bir_lowering=False, debug=False)

    xT_d = nc.declare_dram_parameter("xT", [Cp, Tp], bf, isOutput=False)
    wqk_d = nc.declare_dram_parameter("w_qk", [Cp, 2 * Cp], bf, isOutput=False)
    wv_d = nc.declare_dram_parameter("w_v", [Cp, Cp], bf, isOutput=False)
    wp_d = nc.declare_dram_parameter("w_proj", [Cp, Cp], bf, isOutput=False)
    bqk_d = nc.declare_dram_parameter("b_qk", [M2C, P], f32, isOutput=False)
    bv_d = nc.declare_dram_parameter("b_v", [1, Cp], bf, isOutput=False)
    bp_d = nc.declare_dram_parameter("b_proj", [1, Cp], bf, isOutput=False)
    mask_d = nc.declare_dram_parameter("masks", [DIAG, P, QC], bf, isOutput=False)
    out_d = nc.declare_dram_parameter("out", [Tp, Cp], f32, isOutput=True)

    with tile.TileContext(nc) as tc, ExitStack() as ctx:
        consts = ctx.enter_context(tc.tile_pool(name="consts", bufs=1))
        epool = ctx.enter_context(tc.tile_pool(name="epool", bufs=3))
        rpool = ctx.enter_context(tc.tile_pool(name="rpool", bufs=2))
        psum1 = ctx.enter_context(tc.tile_pool(name="psum1", bufs=4, space="PSUM"))
        psum_s = ctx.enter_context(tc.tile_pool(name="psum_s", bufs=2, space="PSUM"))

        # ---- persistent SBUF buffers ----
        xT = consts.tile([P, CT, Tp], bf)
        wqk = consts.tile([P, CT, 2 * Cp], bf)
        wv = consts.tile([P, CT, Cp], bf)
        wp = consts.tile([P, CT, Cp], bf)
        qkT = consts.tile([P, M2C, Tp], bf)
        # Vp cols 0:HD hold V; cols HD:2HD hold ones, so the AV matmul lands
        # the softmax sums replicated on partitions HD..2HD-1 (no separate
        # PE broadcast needed for the 1/sum normalization).
        Vp = consts.tile([P, TT, Hp, 2 * HDp], bf)
        YT = consts.tile([P, CT, Tp], bf)
        bqk = consts.tile([P, M2C], f32)
        bv = bp = None
        if with_bias_v:
            bv = consts.tile([1, Cp], bf)
        if with_bias_p:
            bp = consts.tile([1, Cp], bf)
        ones = consts.tile([1, max(QC, P)], bf)
        masks = consts.tile([P, DIAG, QC], bf)

        # Chunked input DMAs, ordered by first compute use so the first qk
        # matmul chains can start ~4 MB into the load instead of after the
        # full 10.5 MB (only affects the one-shot path; the reps loop reuses
        # resident weights).
        nc.sync.dma_start(masks[:], mask_d.rearrange("r p q -> p r q"))
        nc.sync.dma_start(bqk[:], bqk_d.rearrange("m p -> p m"))
        if with_bias_v:
            nc.sync.dma_start(bv[:], bv_d[:])
        if with_bias_p:
            nc.sync.dma_start(bp[:], bp_d[:])
        xT_src = xT_d.rearrange("(ct p) t -> p ct t", p=P)
        wqk_src = wqk_d.rearrange("(ct p) n -> p ct n", p=P)
        wv_src = wv_d.rearrange("(ct p) n -> p ct n", p=P)
        wp_src = wp_d.rearrange("(ct p) n -> p ct n", p=P)

        def _dma_wqk_m(m):
            msl = slice(m * P, (m + 1) * P)
            nc.sync.dma_start(wqk[:, :, msl], wqk_src[:, :, msl])

        nc.sync.dma_start(xT[:, :, 0:TCH], xT_src[:, :, 0:TCH])
        for m in (0, M2C // 2):
            _dma_wqk_m(m)
        nc.sync.dma_start(xT[:, :, TCH:], xT_src[:, :, TCH:])
        for m in (1, M2C // 2 + 1):
            _dma_wqk_m(m)
        for vj in range(VJ):
            vsl = slice(vj * QC, (vj + 1) * QC)
            nc.sync.dma_start(wv[:, :, vsl], wv_src[:, :, vsl])
        for u in range(2, M2C // 2):
            _dma_wqk_m(u)
            _dma_wqk_m(M2C // 2 + u)
        for zj in range(VJ):
            zsl = slice(zj * QC, (zj + 1) * QC)
            nc.sync.dma_start(wp[:, :, zsl], wp_src[:, :, zsl])
        nc.gpsimd.memset(ones[:], 1.0)
        nc.gpsimd.memset(Vp[:], 1.0)  # ones columns survive; V region overwritten

        def _emit_qk_chunk(m):
            # qkT[m] = (w_qk^T @ xT)[m] + b_qk : bf16 copyback (DVE)
            msl = slice(m * P, (m + 1) * P)
            for tj in range(TJ):
                tsl = slice(tj * TCH, (tj + 1) * TCH)
                ps = psum1.tile([P, TCH], f32, tag="ps_mm")
                for ct in range(CT):
                    nc.tensor.matmul(
                        ps[:], lhsT=wqk[:, ct, msl], rhs=xT[:, ct, tsl],
                        start=(ct == 0), stop=(ct == CT - 1),
                    )
                if with_bias_qk:
                    nc.vector.tensor_scalar_add(
                        qkT[:, m, tsl], ps[:], bqk[:, m:m + 1],
                    )
                else:
                    nc.scalar.copy(out=qkT[:, m, tsl], in_=ps[:])

        def _emit_v_chunk(ti):
            # V rows [128*ti, 128*(ti+1)) interleaved into Vp[.., ti, h, 0:HD]
            tsl = slice(ti * P, (ti + 1) * P)
            for vj in range(VJ):
                vsl = slice(vj * QC, (vj + 1) * QC)
                ps = psum1.tile([P, QC], f32, tag="ps_mm")
                for ct in range(CT):
                    nc.tensor.matmul(
                        ps[:], lhsT=xT[:, ct, tsl], rhs=wv[:, ct, vsl],
                        start=(ct == 0),
                        stop=(ct == CT - 1 and not with_bias_v),
                    )
                if with_bias_v:
                    nc.tensor.matmul(
                        ps[:], lhsT=ones[0:1, 0:P], rhs=bv[0:1, vsl],
                        start=False, stop=True,
                    )
                hpc = QC // HDp  # heads per chunk
                nc.vector.tensor_copy(
                    out=Vp[:, ti, vj * hpc:(vj + 1) * hpc, vsl_vp],
                    in_=ps[:].rearrange("p (h d) -> p h d", d=HDp),
                )

        def _emit_attn_unit(u, qj):
            # Head pair: head 2u on partitions 0:64, head 2u+1 on 64:128 of
            # qkT chunk u (q) / M2C//2+u (k). The A/B matmuls use disjoint PE
            # row groups (tile_position auto-derived from base_partition), so
            # they run concurrently in the array.
            qk_parts = (
                (qkT[0:HDp, u, :], qkT[0:HDp, M2C // 2 + u, :]),
                (qkT[HDp:P, u, :], qkT[HDp:P, M2C // 2 + u, :]),
            )
            nk = DIAG * (qj + 1)  # active k-tiles (causal)
            q0 = qj * QC
            E_A = epool.tile([P, DIAG * NQ, QC], bf, tag="E")
            E_B = epool.tile([P, DIAG * NQ, QC], bf, tag="E")
            for g in range(nk // 2):
                offg = max(0, P * (2 * g - DIAG * qj))
                ps_h = [
                    psum_s.tile([P, 2, QC], f32, tag="ps_s",
                                name=f"ps_s_{u}_{qj}_{g}_{hh}")
                    for hh in range(2)
                ]
                for r2 in range(2):
                    ki = 2 * g + r2
                    ksl = slice(ki * P, (ki + 1) * P)
                    for half, (qT, kT) in enumerate(qk_parts):
                        nc.tensor.matmul(
                            ps_h[half][:, r2, offg:],
                            lhsT=kT[:, ksl],
                            rhs=qT[:, q0 + offg:q0 + QC],
                            start=True, stop=True,
                        )
                gsl = slice(2 * g, 2 * g + 2)
                nc.scalar.activation(
                    E_A[:, gsl, offg:], ps_h[0][:, :, offg:], AF.Exp)
                nc.scalar.activation(
                    E_B[:, gsl, offg:], ps_h[1][:, :, offg:], AF.Exp)
            # causal mask on the DIAG diagonal tiles (the last ones)
            for rel in range(DIAG):
                ki = DIAG * qj + rel
                off = P * rel
                for E in (E_A, E_B):
                    nc.vector.tensor_mul(
                        out=E[:, ki, off:], in0=E[:, ki, off:],
                        in1=masks[:, rel, off:],
                    )
            # O'[d,q] on partitions 0:HD; softmax sums replicated on
            # partitions HD:2HD via the ones columns of Vp
            pav_A = psum1.tile([P, QC], f32, tag="ps_mm")
            pav_B = psum1.tile([P, QC], f32, tag="ps_mm")
            # sequential per-head chains: consecutive MMs keep the
            # same PSUM bank (alternating banks per-MM causes PE
            # micro-idles / HAM oscillation)
            for pav, E, h in (
                (pav_A, E_A, 2 * u), (pav_B, E_B, 2 * u + 1),
            ):
                for ki in range(nk):
                    off = max(0, P * (ki - DIAG * qj))
                    nc.tensor.matmul(
                        pav[:, off:],
                        lhsT=Vp[:, ki, h, :], rhs=E[:, ki, off:],
                        start=(ki == 0), stop=(ki == nk - 1),
                    )
            # 1/sum on the replicated-sums partitions (64 identical lanes);
            # the iterative DVE reciprocal is ~8 cycles/elem and would
            # dominate the kernel, so use a fast path instead.
            qsl = slice(q0, q0 + QC)
            for pav, half in ((pav_A, 0), (pav_B, 1)):
                rrow = rpool.tile([HDp, QC], f32, tag="rrow")
                if recip_mode == "approx_psum":
                    nc.vector.reciprocal_approx_fast(
                        out=rrow[:], in_=pav[ssl_pav, :])
                elif recip_mode == "approx_sbuf":
                    srow = rpool.tile([HDp, QC], f32, tag="srow")
                    nc.vector.tensor_copy(out=srow[:], in_=pav[ssl_pav, :])
                    nc.vector.reciprocal_approx_fast(out=rrow[:], in_=srow[:])
                elif recip_mode == "lnexp":
                    # r = exp(-ln(s)); Ln and Exp share one ACT table set
                    # (natural_log_exp_and_others), so no extra table loads.
                    srow = rpool.tile([HDp, QC], f32, tag="srow")
                    nc.scalar.activation(srow[:], pav[ssl_pav, :], AF.Ln)
                    nc.scalar.activation(rrow[:], srow[:], AF.Exp, scale=-1.0)
                else:
                    raise ValueError(recip_mode)
                ysl = slice(0, HDp) if half == 0 else slice(HDp, P)
                nc.vector.tensor_mul(
                    out=YT[ysl, u, qsl], in0=pav[osl_pav, :], in1=rrow[:],
                )

        def _emit_proj_chunk(ti):
            # Z rows [128*ti, ..) = Y @ w_proj + b_proj -> DRAM
            tsl = slice(ti * P, (ti + 1) * P)
            for zj in range(VJ):
                zsl = slice(zj * QC, (zj + 1) * QC)
                ps = psum1.tile([P, QC], f32, tag="ps_mm")
                for ct in range(CT):
                    nc.tensor.matmul(
                        ps[:], lhsT=YT[:, ct, tsl], rhs=wp[:, ct, zsl],
                        start=(ct == 0),
                        stop=(ct == CT - 1 and not with_bias_p),
                    )
                if with_bias_p:
                    nc.tensor.matmul(
                        ps[:], lhsT=ones[0:1, 0:P], rhs=bp[0:1, zsl],
                        start=False, stop=True,
                    )
                zt = rpool.tile([P, QC], f32, tag="zt")
                nc.scalar.copy(out=zt[:], in_=ps[:])
                nc.sync.dma_start(out_d[tsl, zsl], zt[:])

        def _emit_body():
            # qkv: emit the q/k chunks for the first attention units early,
            # then V (needed by the first AV), then the rest.
            for u in (0, 1):
                _emit_qk_chunk(u)
                _emit_qk_chunk(M2C // 2 + u)
            for ti in range(TT // 2):
                _emit_v_chunk(ti)
            for u in range(2, Hp // 2):
                _emit_qk_chunk(u)
                _emit_qk_chunk(M2C // 2 + u)
            for ti in range(TT // 2, TT):
                _emit_v_chunk(ti)
            # attention u-outer: each head pair does its small (qj=0,
            # DVE-lean) and large (qj=1, PE-heavy) chunks back to back so
            # the engines stay load-balanced. proj for the first q-half is
            # emitted as soon as its last YT slice exists, giving the
            # scheduler dense PE work while the final unit's exp/normalize
            # run on ACT/DVE.
            for u in range(Hp // 2):
                _emit_attn_unit(u, 0)
                if u == Hp // 2 - 1:
                    for ti in range(QC // P):
                        _emit_proj_chunk(ti)
                _emit_attn_unit(u, 1)
            for ti in range(QC // P, TT):
                _emit_proj_chunk(ti)

        if reps == 1:
            _emit_body()
        else:
            hint = (
                mybir.EngineType.PE,
                mybir.EngineType.DVE,
                mybir.EngineType.Activation,
            )
            with tc.For_i(0, reps, 1, hint_engines=hint):
                _emit_body()

    nc.finalize()
    return nc


def _prep_shared(w_attn, b_attn, w_proj, b_proj):
    """Host-side layout marshalling of the replicated weights (bf16 cast,
    per-head q/k/v column gather, exact 1/8 q pre-scale)."""
    wr = np.asarray(w_attn, np.float32).reshape(C, H, 3, HD)
    w_q = (wr[:, :, 0, :] * np.float32(0.125)).reshape(C, C)
    w_k = wr[:, :, 1, :].reshape(C, C)
    w_qk = np.ascontiguousarray(
        np.concatenate([w_q, w_k], axis=1)
    ).astype(BF16)
    w_v = np.ascontiguousarray(wr[:, :, 2, :].reshape(C, C)).astype(BF16)

    br = np.asarray(b_attn, np.float32).reshape(H, 3, HD)
    # per-partition column layout for the qkT copyback bias: [M2C, 128] f32
    b_qk = np.ascontiguousarray(
        np.concatenate(
            [(br[:, 0, :] * np.float32(0.125)).reshape(C), br[:, 1, :].reshape(C)]
        ).reshape(2 * C // 128, 128)
    )
    b_v = np.ascontiguousarray(br[:, 2, :].reshape(1, C)).astype(BF16)

    wp = np.ascontiguousarray(np.asarray(w_proj, np.float32)).astype(BF16)
    bp = np.ascontiguousarray(np.asarray(b_proj, np.float32).reshape(1, C)).astype(BF16)

    QCv = min(512, T)
    DIAGv = QCv // 128
    k_idx = np.arange(128)[:, None]
    q_idx = np.arange(QCv)[None, :]
    masks = np.stack(
        [(128 * r + k_idx <= q_idx) for r in range(DIAGv)]
    ).astype(BF16)
    return w_qk, w_v, wp, b_qk, b_v, bp, masks


class _Runner:
    """Cached jit(shard_map) executor for a prebuilt Bass module across
    N cores — same lowering as bass2jax.run_bass_via_pjrt, but reusable
    across calls so warm executions can be timed."""

    def __init__(self, nc, n_cores):
        import jax
        import numpy as _np
        from jax.sharding import Mesh, PartitionSpec
        try:
            from jax.experimental.shard_map import shard_map
        except ImportError:
            from jax.shard_map import shard_map
        from concourse import bass2jax, mybir

        bass2jax.install_neuronx_cc_hook()
        assert not nc.dbg_callbacks
        self.dbg_name = nc.dbg_addr.name if nc.dbg_addr is not None else None
        partition_name = (
            nc.partition_id_tensor.name if nc.partition_id_tensor else None
        )

        in_names, out_names, out_avals = [], [], []
        for alloc in nc.m.functions[0].allocations:
            if not isinstance(alloc, mybir.MemoryLocationSet):
                continue
            name = alloc.memorylocations[0].name
            if alloc.kind == "ExternalInput":
                if name != partition_name:
                    in_names.append(name)
            elif alloc.kind == "ExternalOutput":
                out_names.append(name)
                out_avals.append(
                    jax.core.ShapedArray(
                        tuple(alloc.tensor_shape), mybir.dt.np(alloc.dtype)
                    )
                )
        self.n_params = len(in_names)
        self.in_names = list(in_names)
        self.out_names = out_names
        self.out_avals = out_avals
        self.n_cores = n_cores
        all_names = in_names + out_names
        if partition_name is not None:
            all_names = all_names + [partition_name]

        def _body(*args):
            operands = list(args)
            if partition_name is not None:
                operands.append(bass2jax.partition_id_tensor())
            outs = bass2jax._bass_exec_p.bind(
                *operands,
                out_avals=tuple(out_avals),
                in_names=tuple(all_names),
                out_names=tuple(out_names),
                lowering_input_output_aliases=(),
                sim_require_finite=True,
                sim_require_nnan=True,
                nc=nc,
            )
            return tuple(outs)

        devices = jax.devices()[:n_cores]
        mesh = Mesh(_np.asarray(devices), ("core",))
        n_outs = len(out_names)
        # No donation: the kernel writes every element of every output, so
        # the zero "output seed" operands can live on device and be reused
        # across timed calls.
        self.jitted = jax.jit(
            shard_map(
                _body,
                mesh=mesh,
                in_specs=(PartitionSpec("core"),) * (self.n_params + n_outs),
                out_specs=(PartitionSpec("core"),) * n_outs,
                check_rep=False,
            ),
            keep_unused=True,
        )
        from jax.sharding import NamedSharding

        self.sharding = NamedSharding(mesh, PartitionSpec("core"))
        self.dev_zeros = [
            jax.device_put(
                _np.zeros((n_cores * a.shape[0], *a.shape[1:]), a.dtype),
                self.sharding,
            )
            for a in out_avals
        ]

    def prep_args(self, in_maps):
        import jax
        import numpy as _np

        if self.dbg_name is not None:
            # 8-byte PA as uint32[1,2]; zero -> debugger store+halt skipped
            dbg = _np.zeros((1, 2), _np.uint32)
            in_maps = [{**m, self.dbg_name: dbg} for m in in_maps]
        return [
            jax.device_put(
                _np.concatenate(
                    [_np.asarray(m[name]) for m in in_maps], axis=0
                ),
                self.sharding,
            )
            for name in self.in_names
        ]

    def run(self, concat_in):
        import jax

        out = self.jitted(*concat_in, *self.dev_zeros)
        return jax.block_until_ready(out)

    def results(self, out_arrs):
        import numpy as _np

        return [
            {
                name: _np.asarray(out_arrs[i]).reshape(
                    self.n_cores, *self.out_avals[i].shape
                )[c]
                for i, name in enumerate(self.out_names)
            }
            for c in range(self.n_cores)
        ]


_RUNNER_CACHE = {}


def _get_runner(reps=1, with_bias_qk=True, with_bias_v=True, with_bias_p=True):
    key = (T, C, H, HD, reps, with_bias_qk, with_bias_v, with_bias_p,
           RECIP_MODE, VP_ORDER)
    if key not in _RUNNER_CACHE:
        if key not in _NC_CACHE:
            _NC_CACHE[key] = _build_nc(
                T, C, H, HD, reps=reps, with_bias_qk=with_bias_qk,
                with_bias_v=with_bias_v, with_bias_p=with_bias_p,
            )
        _RUNNER_CACHE[key] = _Runner(_NC_CACHE[key], N_CORES)
    return _RUNNER_CACHE[key]


def _make_in_maps(x, w_attn, b_attn, w_proj, b_proj):
    w_qk, w_v, wp, b_qk, b_v, bp, masks = _prep_shared(
        w_attn, b_attn, w_proj, b_proj
    )
    x = np.asarray(x, np.float32)
    in_maps = []
    for i in range(N_CORES):
        xT = np.ascontiguousarray(x[i].T).astype(BF16)
        in_maps.append({
            "xT": xT, "w_qk": w_qk, "w_v": w_v, "w_proj": wp,
            "b_qk": b_qk, "b_v": b_v, "b_proj": bp, "masks": masks,
        })
    return in_maps


def _bias_flags(b_attn, b_proj):
    br = np.asarray(b_attn, np.float32).reshape(H, 3, HD)
    return (
        bool(br[:, 0:2, :].any()),
        bool(br[:, 2, :].any()),
        bool(np.asarray(b_proj, np.float32).any()),
    )


def kernel(x, w_attn, b_attn, w_proj, b_proj):
    wbqk, wbv, wbp = _bias_flags(b_attn, b_proj)
    runner = _get_runner(with_bias_qk=wbqk, with_bias_v=wbv, with_bias_p=wbp)
    concat_in = runner.prep_args(
        _make_in_maps(x, w_attn, b_attn, w_proj, b_proj)
    )
    res = runner.results(runner.run(concat_in))
    return np.stack([res[i]["out"] for i in range(N_CORES)]).astype(np.float32)


def measure_pair(x, w_attn, b_attn, w_proj, b_proj, iters=12, reps=50):
    """Interleaved warm wall-clock times (s) for reps=1 and reps=R bodies.

    Returns (t1, tR) lists. Interleaving makes the pairwise delta robust
    to slow drift in the axon dispatch RTT."""
    import time

    wbqk, wbv, wbp = _bias_flags(b_attn, b_proj)
    r1 = _get_runner(reps=1, with_bias_qk=wbqk, with_bias_v=wbv,
                     with_bias_p=wbp)
    rR = _get_runner(reps=reps, with_bias_qk=wbqk, with_bias_v=wbv,
                     with_bias_p=wbp)
    in1 = r1.prep_args(_make_in_maps(x, w_attn, b_attn, w_proj, b_proj))
    inR = rR.prep_args(_make_in_maps(x, w_attn, b_attn, w_proj, b_proj))
    r1.run(in1)
    rR.run(inR)
    t1, tR = [], []
    for _ in range(iters):
        t0 = time.perf_counter()
        r1.run(in1)
        t1.append(time.perf_counter() - t0)
        t0 = time.perf_counter()
        rR.run(inR)
        tR.append(time.perf_counter() - t0)
    return t1, tR


def measure(x, w_attn, b_attn, w_proj, b_proj, iters=5, reps=1):
    """Warm wall-clock times (s) of the sharded on-device execution.

    reps > 1 uses a kernel variant whose compute body runs `reps` times
    per dispatch (device-side loop), so per-iteration HW time can be
    resolved despite the ~90 ms axon round-trip overhead."""
    import time

    wbqk, wbv, wbp = _bias_flags(b_attn, b_proj)
    runner = _get_runner(
        reps=reps, with_bias_qk=wbqk, with_bias_v=wbv, with_bias_p=wbp,
    )
    concat_in = runner.prep_args(
        _make_in_maps(x, w_attn, b_attn, w_proj, b_proj)
    )
    runner.run(concat_in)  # warm-up / compile
    times = []
    for _ in range(iters):
        t0 = time.perf_counter()
        runner.run(concat_in)
        times.append(time.perf_counter() - t0)
    return times

